# revision 1
# baseline (speedup 1.0000x reference)
"""Trainium2 Bass kernel for nn_CellAnnotator (per-pixel 8x8 locally-connected
weighted pooling with normalization), SPMD across 8 NeuronCores.

Contract: kernel(**inputs) takes FULL inputs (x0 [512,512,128] f32,
weights [512,512,64] f32, cnts [512,512,1] f32) and returns the FULL
output [512,512,128] f32.

Sharding: rows (H) split across 8 cores, 64 output rows each; each core's
input shard carries a 3+4-row halo (built host-side, zero-padded at the
image borders), so no device-to-device communication is needed.

Algorithm (banded matmul on the TensorEngine):
  out[i,j,c] = sum_{p,q} w[i,j,p*8+q] * x_pad[i+p, j+q, c], normalized by
  the same pooling applied to cnts (appended as channel 128 of x).
  For an output row i and a 57-pixel column block, the 64-tap sum is done as
  4 PSUM-accumulated matmuls, one per input-row pair: contraction dim
  K = 128 = (2 rows) x (64 input cols); lhsT is a banded [128, 57] weight
  tile (built host-side: w[i,j,p*8+q] sits at [64*(p%2) + (j-j0) + q, j-j0]);
  rhs is the bf16 input tile [128, 129] (128 channels + cnts).

DMA batching: input tiles for one row-pair are loaded for all 9 column
blocks in 2 DMAs; band tiles come as one 525KB DMA per row; the output row
is staged in one SBUF tile (blocked pixel layout, de-blocked on host) and
stored with a single DMA.
"""

import numpy as np
import ml_dtypes
from contextlib import ExitStack

import concourse.bass as bass
import concourse.bacc as bacc
import concourse.mybir as mybir
import concourse.tile as tile
from concourse.ap import AP
from concourse.bass_utils import run_bass_kernel_spmd

BF16 = np.dtype(ml_dtypes.bfloat16)

# Problem constants (hardcoded per contract)
H, W, C = 512, 512, 128
ROI = 8
TAPS = ROI * ROI
PAD_LO, PAD_HI = 3, 4          # XLA SAME padding for even kernel
NCORES = 8
ROWS = H // NCORES             # 64 output rows per core
IN_ROWS = ROWS + ROI - 1       # 71 input rows (halo included)
WPAD = W + ROI                 # padded width: cols -3 .. 516 (520)
CCH = C + 1                    # x channels + cnts as channel 128

BLK = 57                       # output pixels per column block
NBLK = 9                       # 8*57 + 56 = 512
PPAIRS = 4                     # input-row pairs per output row
BFREE = PPAIRS * NBLK * BLK    # free size of one row's band line (2052)

_CACHE = {}


def _build_nc(rep=1, variant="full"):
    f32 = mybir.dt.float32
    bf = mybir.dt.bfloat16
    nc = bacc.Bacc("TRN2", target_bir_lowering=False, debug=False,
                   num_devices=NCORES)
    # pre-stacked pair tiles: xcp[ri, 64g+u, b, c] = xpad[ri+g, 57b + u, c]
    NPAIRS = IN_ROWS - 1
    xcp = nc.dram_tensor("xcp", [NPAIRS, 128, NBLK, CCH], bf,
                         kind="ExternalInput")
    bnd = nc.dram_tensor("bnd", [ROWS, 128, BFREE], bf, kind="ExternalInput")
    # blocked output layout: [row, jj, b, c]; de-blocked + upcast host-side
    out = nc.dram_tensor("out", [ROWS, BLK, NBLK, C], bf,
                         kind="ExternalOutput")

    with tile.TileContext(nc) as tc:
        with ExitStack() as ctx:
            if rep > 1:
                ctx.enter_context(tc.For_i(0, rep, 1))
            xpool = ctx.enter_context(tc.tile_pool(name="xp", bufs=12))
            bpool = ctx.enter_context(tc.tile_pool(name="bp", bufs=3))
            ppool = ctx.enter_context(
                tc.tile_pool(name="pp", bufs=8, space="PSUM"))
            opool = ctx.enter_context(tc.tile_pool(name="op", bufs=3))
            spool = ctx.enter_context(tc.tile_pool(name="sp", bufs=4))

            xcache = {}

            def get_x(ri):
                """Input tiles for row pair (ri, ri+1), all column blocks:
                [128 = 2x64 positions, 9 blocks, 129 channels]."""
                if ri not in xcache:
                    t = xpool.tile([128, NBLK, CCH], bf, tag="xt")
                    fr = NBLK * CCH
                    src = AP(xcp, ri * 128 * fr, [[fr, 128], [1, fr]])
                    nc.sync.dma_start(t[:], src)
                    xcache[ri] = t
                return xcache[ri]

            if variant in ("dmax", "dmao", "dmao4"):
                # X-only / out-only DMA probes
                if variant == "dmao4":
                    ot4 = opool.tile([BLK, 4, NBLK * C], bf, tag="ot4")
                    nc.vector.memset(ot4[:], 0.0)
                    fo = NBLK * C
                    for il in range(0, ROWS, 4):
                        dst = AP(out, il * BLK * fo,
                                 [[fo, BLK], [BLK * fo, 4], [1, fo]])
                        nc.sync.dma_start(dst, ot4[:])
                else:
                    ot0 = opool.tile([BLK, NBLK, C], bf, tag="ot")
                    nc.vector.memset(ot0[:], 0.0)
                    for il in range(ROWS):
                        if variant == "dmax":
                            for pp in range(PPAIRS):
                                get_x(il + 2 * pp)
                        else:
                            nc.sync.dma_start(out[il], ot0[:])
                _finish = True
            else:
                _finish = False
            OBATCH = 4
            fo = NBLK * C
            otile = None
            btile0 = None
            if variant == "nodma":
                btile0 = bpool.tile([128, PPAIRS, NBLK, BLK], bf, tag="bt")
                nc.sync.dma_start(
                    btile0[:], AP(bnd, 0, [[BFREE, 128], [1, BFREE]]))
            for il in range(ROWS if not _finish else 0):
                if variant == "nodma":
                    btile = btile0
                else:
                    btile = bpool.tile([128, PPAIRS, NBLK, BLK], bf,
                                       tag="bt")
                    nc.sync.dma_start(
                        btile[:],
                        AP(bnd, il * 128 * BFREE, [[BFREE, 128], [1, BFREE]]))
                if variant == "dmab":
                    continue
                if il % OBATCH == 0:
                    otile = opool.tile([BLK, OBATCH, fo], bf, tag="ot")
                if variant == "dma":
                    for pp in range(PPAIRS):
                        get_x(il + 2 * pp)
                    continue
                for b in range(NBLK):
                    m = W - BLK * b if b == NBLK - 1 else BLK
                    psum = ppool.tile([BLK, CCH], f32, tag="ps")
                    for pp in range(PPAIRS):
                        xt = get_x((il + 2 * pp) % 8 if variant == "nodma"
                                   else il + 2 * pp)
                        nc.tensor.matmul(
                            psum[:m, :], btile[:, pp, b, :m], xt[:, b, :],
                            start=(pp == 0), stop=(pp == PPAIRS - 1))
                    if variant == "pe":
                        nc.vector.tensor_copy(otile[:m, b, :], psum[:m, 0:C])
                        continue
                    rec = spool.tile([BLK, 1], f32, tag="rec")
                    nc.vector.tensor_scalar_add(
                        rec[:m, :], psum[:m, C:C + 1], 1e-6)
                    nc.vector.reciprocal(rec[:m, :], rec[:m, :])
                    odst = otile[:m, il % OBATCH, b * C:(b + 1) * C]
                    if b % 2 == 0:
                        nc.vector.tensor_scalar(
                            odst, psum[:m, 0:C], rec[:m, 0:1], None,
                            op0=mybir.AluOpType.mult)
                    else:
                        nc.scalar.activation(
                            odst, psum[:m, 0:C],
                            mybir.ActivationFunctionType.Identity,
                            scale=rec[:m, 0:1])
                if il % OBATCH == OBATCH - 1:
                    dst = AP(out, (il - OBATCH + 1) * BLK * fo,
                             [[fo, BLK], [BLK * fo, OBATCH], [1, fo]])
                    nc.scalar.dma_start(dst, otile[:])
    nc.compile()
    return nc


def _get_nc(rep=1, variant="full"):
    key = ("nc", rep, variant)
    if key not in _CACHE:
        _CACHE[key] = _build_nc(rep, variant)
    return _CACHE[key]


def _build_bands(weights):
    """bands[i, kpos, pp, b, jj] = w[i, 57b+jj, (2pp+g)*8 + d] at
    kpos = 64g + jj + d  (g = kpos//64, d in [0,8)); zero elsewhere."""
    wq = weights.reshape(H, W, ROI, ROI)          # [i, j, p, q]
    bands = np.zeros((H, 128, PPAIRS, NBLK, BLK), BF16)
    for b in range(NBLK):
        m = W - BLK * b if b == NBLK - 1 else BLK
        jv = np.arange(m)
        wb = wq[:, BLK * b:BLK * b + m]            # [H, m, 8, 8]
        for pp in range(PPAIRS):
            for g in range(2):
                p = 2 * pp + g
                for d in range(ROI):
                    bands[:, 64 * g + jv + d, pp, b, jv] = \
                        wb[:, jv, p, d].astype(BF16)
    return bands


def _host_prep(x0, weights, cnts):
    xcp = np.zeros((H + ROI - 1, WPAD, CCH), BF16)
    xcp[PAD_LO:PAD_LO + H, PAD_LO:PAD_LO + W, :C] = x0.astype(BF16)
    xcp[PAD_LO:PAD_LO + H, PAD_LO:PAD_LO + W, C] = cnts[:, :, 0].astype(BF16)
    # pre-stacked pair tiles: xb[ri, 64g+u, b, c] = xcp[ri+g, 57b + u, c]
    # (overlapping 64-wide windows at stride 57; rows duplicated so each
    # pair tile is one fully-contiguous DMA)
    sr, sc, sch = xcp.strides
    xb = np.lib.stride_tricks.as_strided(
        xcp, shape=(H + ROI - 2, 2, 64, NBLK, CCH),
        strides=(sr, sr, sc, BLK * sc, sch)).reshape(
            H + ROI - 2, 128, NBLK, CCH)
    bands = _build_bands(weights)
    in_maps = []
    for k in range(NCORES):
        r0 = k * ROWS
        in_maps.append({
            "xcp": np.ascontiguousarray(xb[r0:r0 + IN_ROWS - 1]),
            "bnd": np.ascontiguousarray(
                bands[r0:r0 + ROWS].reshape(ROWS, 128, BFREE)),
        })
    return in_maps


def _unblock(arr):
    """[ROWS, BLK, NBLK, C] blocked -> [ROWS, W, C] f32."""
    return arr.astype(np.float32).transpose(0, 2, 1, 3).reshape(
        ROWS, NBLK * BLK, C)[:, :W]


def kernel(x0, weights, cnts):
    x0 = np.asarray(x0, np.float32)
    weights = np.asarray(weights, np.float32)
    cnts = np.asarray(cnts, np.float32)
    nc = _get_nc()
    in_maps = _host_prep(x0, weights, cnts)
    res = run_bass_kernel_spmd(nc, in_maps, core_ids=list(range(NCORES)))
    return np.ascontiguousarray(np.concatenate(
        [_unblock(res.results[k]["out"]) for k in range(NCORES)], axis=0))



# revision 6
# speedup vs baseline: 1.1447x; 1.1447x over previous
"""Trainium2 Bass kernel for nn_CellAnnotator (per-pixel 8x8 locally-connected
weighted pooling with normalization), SPMD across 8 NeuronCores.

Contract: kernel(**inputs) takes FULL inputs (x0 [512,512,128] f32,
weights [512,512,64] f32, cnts [512,512,1] f32) and returns the FULL
output [512,512,128] f32.

Sharding: rows (H) split across 8 cores, 64 output rows each; each core's
input shard carries a 3+4-row halo (built host-side, zero-padded at the
image borders), so no device-to-device communication is needed.

v2 design:
- The normalization denominator att(cnts, weights) depends only on the
  inputs, so it is folded into the weights host-side:
  w' = w / (att(cnts, w) + 1e-6); then out = att(x0, w') exactly.
  The device runs pure pooling: no cnts channel, no reciprocal/multiply.
- Banded matmul on the TensorEngine: for output row i and a 57-pixel
  column block, the 64-tap sum is PSUM-accumulated over input-row pairs:
  K = 128 = (2 rows) x (64 input cols); lhsT is a banded [128, 57] w'
  tile; rhs is the bf16 input-pair tile [128, 128].
- Even-pair x sharing: x pair tiles exist only for even ri (rows ri,ri+1),
  so each input row is loaded once (plus the 64/57 column overlap). Even
  output rows accumulate 4 pairs; odd rows 5 pairs (the first/last pair
  contributes one row; its other half of the band is zero).
- DMA batching: x tiles hold 2 even pairs (18 DMAs on the sync queue);
  band tiles hold one even+odd row pair of bands, 4+5 pair-slots
  (32 DMAs on the gpsimd queue); outputs are staged 4 rows per tile
  (16 DMAs on the scalar queue).
"""

import numpy as np
import ml_dtypes
from contextlib import ExitStack

import concourse.bass as bass
import concourse.bacc as bacc
import concourse.mybir as mybir
import concourse.tile as tile
from concourse.ap import AP
from concourse.bass_utils import run_bass_kernel_spmd

BF16 = np.dtype(ml_dtypes.bfloat16)

# Problem constants (hardcoded per contract)
H, W, C = 512, 512, 128
ROI = 8
PAD_LO, PAD_HI = 3, 4          # XLA SAME padding for even kernel
NCORES = 8
ROWS = H // NCORES             # 64 output rows per core
IN_ROWS = ROWS + ROI           # 72 input rows (halo + even-pair pad)
WPAD = W + ROI                 # padded width: cols -3 .. 516 (520)

BLK = 57                       # output pixels per column block
NBLK = 9                       # 8*57 + 56 = 512
NXT = 18                       # x tiles: 2 even pairs each (ri = 4t, 4t+2)
SLOTS = 9                      # band pair-slots per (even,odd) row pair: 4+5
BFREE = SLOTS * NBLK * BLK     # band free elems per row-pair line (4617)

_CACHE = {}


def _build_nc(rep=1):
    f32 = mybir.dt.float32
    bf = mybir.dt.bfloat16
    nc = bacc.Bacc("TRN2", target_bir_lowering=False, debug=False,
                   num_devices=NCORES)
    # x pair tiles (even ri only), 2 pairs per line:
    # xcp[t, 64g+u, e, b, c] = xpad[4t+2e+g, 57b+u, c]
    xcp = nc.dram_tensor("xcp", [NXT, 128, 2, NBLK, C], bf,
                         kind="ExternalInput")
    # bands for row pair (2r, 2r+1): 4 even slots + 5 odd slots
    bnd = nc.dram_tensor("bnd", [ROWS // 2, 128, BFREE], bf,
                         kind="ExternalInput")
    # blocked output layout: [row, jj, b, c]; de-blocked + upcast host-side
    out = nc.dram_tensor("out", [ROWS, BLK, NBLK, C], bf,
                         kind="ExternalOutput")

    with tile.TileContext(nc) as tc:
        with ExitStack() as ctx:
            if rep > 1:
                ctx.enter_context(tc.For_i(0, rep, 1))
            xpool = ctx.enter_context(tc.tile_pool(name="xp", bufs=8))
            bpool = ctx.enter_context(tc.tile_pool(name="bp", bufs=4))
            ppool = ctx.enter_context(
                tc.tile_pool(name="pp", bufs=8, space="PSUM"))
            opool = ctx.enter_context(tc.tile_pool(name="op", bufs=3))

            xcache = {}

            def get_x(t):
                """Two even input pairs (ri = 4t, 4t+2), all column blocks:
                [128 = 2x64 positions, 2 pairs, 9 blocks, 128 channels]."""
                if t not in xcache:
                    xt = xpool.tile([128, 2, NBLK, C], bf, tag="xt")
                    fr = 2 * NBLK * C
                    src = AP(xcp, t * 128 * fr, [[fr, 128], [1, fr]])
                    nc.sync.dma_start(xt[:], src)
                    xcache[t] = xt
                return xcache[t]

            OBATCH = 4
            fo = NBLK * C
            otile = None
            btile = None
            for il in range(ROWS):
                if il % 2 == 0:
                    btile = bpool.tile([128, SLOTS, NBLK, BLK], bf, tag="bt")
                    nc.gpsimd.dma_start(
                        btile[:],
                        AP(bnd, (il // 2) * 128 * BFREE,
                           [[BFREE, 128], [1, BFREE]]))
                if il % OBATCH == 0:
                    otile = opool.tile([BLK, OBATCH, fo], bf, tag="ot")
                if il % 2 == 0:
                    pairs = [il, il + 2, il + 4, il + 6]
                    slot0 = 0
                else:
                    pairs = [il - 1, il + 1, il + 3, il + 5, il + 7]
                    slot0 = 4
                for b in range(NBLK):
                    m = W - BLK * b if b == NBLK - 1 else BLK
                    psum = ppool.tile([BLK, C], f32, tag="ps")
                    for s, ri in enumerate(pairs):
                        xt = get_x(ri // 4)
                        nc.tensor.matmul(
                            psum[:m, :], btile[:, slot0 + s, b, :m],
                            xt[:, (ri % 4) // 2, b, :],
                            start=(s == 0), stop=(s == len(pairs) - 1))
                    nc.vector.tensor_copy(
                        otile[:m, il % OBATCH, b * C:(b + 1) * C],
                        psum[:m, :])
                if il % OBATCH == OBATCH - 1:
                    dst = AP(out, (il - OBATCH + 1) * BLK * fo,
                             [[fo, BLK], [BLK * fo, OBATCH], [1, fo]])
                    nc.scalar.dma_start(dst, otile[:])
    nc.compile()
    return nc


def _get_nc(rep=1, variant="full"):
    key = ("nc", rep, variant)
    if key not in _CACHE:
        _CACHE[key] = _build_nc(rep)
    return _CACHE[key]


def _total_cnts(weights, cnts):
    """att(cnts, weights): [H, W] pooled counts (reference padding)."""
    cp = np.zeros((H + ROI - 1, W + ROI - 1), np.float32)
    cp[PAD_LO:PAD_LO + H, PAD_LO:PAD_LO + W] = cnts[:, :, 0]
    wq = weights.reshape(H, W, ROI, ROI)
    tot = np.zeros((H, W), np.float32)
    for p in range(ROI):
        for q in range(ROI):
            tot += cp[p:p + H, q:q + W] * wq[:, :, p, q]
    return tot


def _build_bands(wn):
    """Banded layout of prenormalized weights, 2 output rows per line.

    bands[r, kpos, slot, b, jj] = wn[i, 57b+jj, p, d] at kpos = 64g+jj+d,
    where for even row i=2r: slot in [0,4), p = 2*slot + g;
    for odd row i=2r+1: slot in [4,9), p = 2*(slot-4) - 1 + g, skipping
    p outside [0,8). Zero elsewhere.
    """
    wq = wn.reshape(H, W, ROI, ROI)
    bands = np.zeros((H // 2, 128, SLOTS, NBLK, BLK), BF16)
    for b in range(NBLK):
        m = W - BLK * b if b == NBLK - 1 else BLK
        jv = np.arange(m)
        for parity in (0, 1):
            rows = np.arange(parity, H, 2)
            wb = wq[rows, BLK * b:BLK * b + m]     # [H/2, m, 8, 8]
            nslot = 4 if parity == 0 else 5
            for s in range(nslot):
                for g in range(2):
                    p = 2 * s + g if parity == 0 else 2 * s - 1 + g
                    if p < 0 or p >= ROI:
                        continue
                    slot = s if parity == 0 else 4 + s
                    for d in range(ROI):
                        bands[:, 64 * g + jv + d, slot, b, jv] = \
                            wb[:, jv, p, d].astype(BF16)
    return bands


def _host_prep(x0, weights, cnts):
    tot = _total_cnts(weights, cnts) + 1e-6
    wn = weights / tot[:, :, None]

    # padded image: row pr holds x0[pr-3]; rows 0-2 and 515-519 zero.
    # Output global row gi reads padded rows gi..gi+7; core k's local input
    # row lr is padded row r0+lr, lr in [0, 72).
    xpad = np.zeros((H + ROI, WPAD, C), BF16)
    xpad[PAD_LO:PAD_LO + H, PAD_LO:PAD_LO + W, :] = x0.astype(BF16)
    bands = _build_bands(wn)
    in_maps = []
    for k in range(NCORES):
        r0 = k * ROWS
        # xcp[t, 64g+u, e, b, c] = xloc[4t+2e+g, 57b+u, c]
        xloc = xpad[r0:r0 + IN_ROWS]
        s0, s1, s2 = xloc.strides
        xb = np.lib.stride_tricks.as_strided(
            xloc, shape=(NXT, 2, 64, 2, NBLK, C),
            strides=(4 * s0, s0, s1, 2 * s0, BLK * s1, s2))
        # xb[t, g, u, e, b, c] -> want [t, 64g+u, e, b, c]
        xb = xb.reshape(NXT, 128, 2, NBLK, C)
        in_maps.append({
            "xcp": np.ascontiguousarray(xb),
            "bnd": np.ascontiguousarray(
                bands[r0 // 2:(r0 + ROWS) // 2].reshape(
                    ROWS // 2, 128, BFREE)),
        })
    return in_maps


def _unblock(arr):
    """[ROWS, BLK, NBLK, C] blocked -> [ROWS, W, C] f32."""
    return arr.astype(np.float32).transpose(0, 2, 1, 3).reshape(
        ROWS, NBLK * BLK, C)[:, :W]


def kernel(x0, weights, cnts):
    x0 = np.asarray(x0, np.float32)
    weights = np.asarray(weights, np.float32)
    cnts = np.asarray(cnts, np.float32)
    nc = _get_nc()
    in_maps = _host_prep(x0, weights, cnts)
    res = run_bass_kernel_spmd(nc, in_maps, core_ids=list(range(NCORES)))
    return np.ascontiguousarray(np.concatenate(
        [_unblock(res.results[k]["out"]) for k in range(NCORES)], axis=0))


# revision 7
# speedup vs baseline: 1.3800x; 1.2055x over previous
"""Trainium2 Bass kernel for nn_CellAnnotator (per-pixel 8x8 locally-connected
weighted pooling with normalization), SPMD across 8 NeuronCores.

Contract: kernel(**inputs) takes FULL inputs (x0 [512,512,128] f32,
weights [512,512,64] f32, cnts [512,512,1] f32) and returns the FULL
output [512,512,128] f32.

Sharding: rows (H) split across 8 cores, 64 output rows each; each core's
input shard carries a 3+4-row halo (built host-side, zero-padded at the
image borders), so no device-to-device communication is needed.

v2 design:
- The normalization denominator att(cnts, weights) depends only on the
  inputs, so it is folded into the weights host-side:
  w' = w / (att(cnts, w) + 1e-6); then out = att(x0, w') exactly.
  The device runs pure pooling: no cnts channel, no reciprocal/multiply.
- Banded matmul on the TensorEngine: for output row i and a 57-pixel
  column block, the 64-tap sum is PSUM-accumulated over input-row pairs:
  K = 128 = (2 rows) x (64 input cols); lhsT is a banded [128, 57] w'
  tile; rhs is the bf16 input-pair tile [128, 128].
- Even-pair x sharing: x pair tiles exist only for even ri (rows ri,ri+1),
  so each input row is loaded once (plus the 64/57 column overlap). Even
  output rows accumulate 4 pairs; odd rows 5 pairs (the first/last pair
  contributes one row; its other half of the band is zero).
- DMA batching: x tiles hold 2 even pairs (18 DMAs on the sync queue);
  band tiles hold one even+odd row pair of bands, 4+5 pair-slots
  (32 DMAs on the gpsimd queue); outputs are staged 4 rows per tile
  (16 DMAs on the scalar queue).
"""

import numpy as np
import ml_dtypes
from contextlib import ExitStack

import concourse.bass as bass
import concourse.bacc as bacc
import concourse.mybir as mybir
import concourse.tile as tile
from concourse.ap import AP
from concourse.bass_utils import run_bass_kernel_spmd

BF16 = np.dtype(ml_dtypes.bfloat16)
BF8E3 = np.dtype(ml_dtypes.float8_e3m4)   # E3M4: 4 mantissa bits, max 15.5

# Problem constants (hardcoded per contract)
H, W, C = 512, 512, 128
ROI = 8
PAD_LO, PAD_HI = 3, 4          # XLA SAME padding for even kernel
NCORES = 8
ROWS = H // NCORES             # 64 output rows per core
IN_ROWS = ROWS + ROI           # 72 input rows (halo + even-pair pad)
WPAD = W + ROI                 # padded width: cols -3 .. 516 (520)

BLK = 57                       # output pixels per column block
NBLK = 9                       # 8*57 + 56 = 512
NXT = 18                       # x tiles: 2 even pairs each (ri = 4t, 4t+2)
SLOTS = 9                      # band pair-slots per (even,odd) row pair: 4+5
BFREE = SLOTS * NBLK * BLK     # band free elems per row-pair line (4617)

_CACHE = {}


def _build_nc(rep=1):
    f32 = mybir.dt.float32
    bf = mybir.dt.bfloat16
    f8 = mybir.dt.float8e3
    nc = bacc.Bacc("TRN2", target_bir_lowering=False, debug=False,
                   num_devices=NCORES)
    # x pair tiles (even ri only), 2 pairs per line:
    # xcp[t, 64g+u, e, b, c] = xpad[4t+2e+g, 57b+u, c]
    xcp = nc.dram_tensor("xcp", [NXT, 128, 2, NBLK, C], bf,
                         kind="ExternalInput")
    # bands for row pair (2r, 2r+1): 4 even slots + 5 odd slots
    bnd = nc.dram_tensor("bnd", [ROWS // 2, 128, BFREE], f8,
                         kind="ExternalInput")
    # blocked output layout: [row, jj, b, c]; de-blocked + upcast host-side
    out = nc.dram_tensor("out", [ROWS, BLK, NBLK, C], bf,
                         kind="ExternalOutput")

    with tile.TileContext(nc) as tc:
        with ExitStack() as ctx:
            if rep > 1:
                ctx.enter_context(tc.For_i(0, rep, 1))
            xpool = ctx.enter_context(tc.tile_pool(name="xp", bufs=8))
            bpool = ctx.enter_context(tc.tile_pool(name="bp", bufs=4))
            ppool = ctx.enter_context(
                tc.tile_pool(name="pp", bufs=8, space="PSUM"))
            opool = ctx.enter_context(tc.tile_pool(name="op", bufs=3))

            xcache = {}

            def get_x(t):
                """Two even input pairs (ri = 4t, 4t+2), all column blocks:
                [128 = 2x64 positions, 2 pairs, 9 blocks, 128 channels]."""
                if t not in xcache:
                    xt = xpool.tile([128, 2, NBLK, C], bf, tag="xt")
                    fr = 2 * NBLK * C
                    src = AP(xcp, t * 128 * fr, [[fr, 128], [1, fr]])
                    nc.sync.dma_start(xt[:], src)
                    xcache[t] = xt
                return xcache[t]

            OBATCH = 4
            fo = NBLK * C
            otile = None
            btile = None
            for il in range(ROWS):
                if il % 2 == 0:
                    btile = bpool.tile([128, SLOTS, NBLK, BLK], f8, tag="bt")
                    nc.gpsimd.dma_start(
                        btile[:],
                        AP(bnd, (il // 2) * 128 * BFREE,
                           [[BFREE, 128], [1, BFREE]]))
                if il % OBATCH == 0:
                    otile = opool.tile([BLK, OBATCH, fo], bf, tag="ot")
                if il % 2 == 0:
                    pairs = [il, il + 2, il + 4, il + 6]
                    slot0 = 0
                else:
                    pairs = [il - 1, il + 1, il + 3, il + 5, il + 7]
                    slot0 = 4
                for b in range(NBLK):
                    m = W - BLK * b if b == NBLK - 1 else BLK
                    psum = ppool.tile([BLK, C], f32, tag="ps")
                    for s, ri in enumerate(pairs):
                        xt = get_x(ri // 4)
                        nc.tensor.matmul(
                            psum[:m, :], btile[:, slot0 + s, b, :m],
                            xt[:, (ri % 4) // 2, b, :],
                            start=(s == 0), stop=(s == len(pairs) - 1))
                    nc.vector.tensor_copy(
                        otile[:m, il % OBATCH, b * C:(b + 1) * C],
                        psum[:m, :])
                if il % OBATCH == OBATCH - 1:
                    dst = AP(out, (il - OBATCH + 1) * BLK * fo,
                             [[fo, BLK], [BLK * fo, OBATCH], [1, fo]])
                    nc.scalar.dma_start(dst, otile[:])
    nc.compile()
    return nc


def _get_nc(rep=1, variant="full"):
    key = ("nc", rep, variant)
    if key not in _CACHE:
        _CACHE[key] = _build_nc(rep)
    return _CACHE[key]


def _total_cnts(weights, cnts):
    """att(cnts, weights): [H, W] pooled counts (reference padding)."""
    cp = np.zeros((H + ROI - 1, W + ROI - 1), np.float32)
    cp[PAD_LO:PAD_LO + H, PAD_LO:PAD_LO + W] = cnts[:, :, 0]
    wq = weights.reshape(H, W, ROI, ROI)
    tot = np.zeros((H, W), np.float32)
    for p in range(ROI):
        for q in range(ROI):
            tot += cp[p:p + H, q:q + W] * wq[:, :, p, q]
    return tot


def _build_bands(wn):
    """Banded layout of prenormalized weights, 2 output rows per line.

    bands[r, kpos, slot, b, jj] = wn[i, 57b+jj, p, d] at kpos = 64g+jj+d,
    where for even row i=2r: slot in [0,4), p = 2*slot + g;
    for odd row i=2r+1: slot in [4,9), p = 2*(slot-4) - 1 + g, skipping
    p outside [0,8). Zero elsewhere.
    """
    wq = wn.reshape(H, W, ROI, ROI)
    bands = np.zeros((H // 2, 128, SLOTS, NBLK, BLK), BF8E3)
    for b in range(NBLK):
        m = W - BLK * b if b == NBLK - 1 else BLK
        jv = np.arange(m)
        for parity in (0, 1):
            rows = np.arange(parity, H, 2)
            wb = wq[rows, BLK * b:BLK * b + m]     # [H/2, m, 8, 8]
            nslot = 4 if parity == 0 else 5
            for s in range(nslot):
                for g in range(2):
                    p = 2 * s + g if parity == 0 else 2 * s - 1 + g
                    if p < 0 or p >= ROI:
                        continue
                    slot = s if parity == 0 else 4 + s
                    for d in range(ROI):
                        bands[:, 64 * g + jv + d, slot, b, jv] = \
                            wb[:, jv, p, d].astype(BF8E3)
    return bands


def _host_prep(x0, weights, cnts):
    tot = _total_cnts(weights, cnts) + 1e-6
    wn = weights / tot[:, :, None]
    # scale bands into E3M4's sweet range [~2^-6, 15.5]; fold the inverse
    # scale into x via an exact bf16 exponent shift
    k = int(np.floor(np.log2(15.5 / wn.max())))
    wn = wn * np.float32(2.0 ** k)
    x0 = x0 * np.float32(2.0 ** -k)

    # padded image: row pr holds x0[pr-3]; rows 0-2 and 515-519 zero.
    # Output global row gi reads padded rows gi..gi+7; core k's local input
    # row lr is padded row r0+lr, lr in [0, 72).
    xpad = np.zeros((H + ROI, WPAD, C), BF16)
    xpad[PAD_LO:PAD_LO + H, PAD_LO:PAD_LO + W, :] = x0.astype(BF16)
    bands = _build_bands(wn)
    in_maps = []
    for k in range(NCORES):
        r0 = k * ROWS
        # xcp[t, 64g+u, e, b, c] = xloc[4t+2e+g, 57b+u, c]
        xloc = xpad[r0:r0 + IN_ROWS]
        s0, s1, s2 = xloc.strides
        xb = np.lib.stride_tricks.as_strided(
            xloc, shape=(NXT, 2, 64, 2, NBLK, C),
            strides=(4 * s0, s0, s1, 2 * s0, BLK * s1, s2))
        # xb[t, g, u, e, b, c] -> want [t, 64g+u, e, b, c]
        xb = xb.reshape(NXT, 128, 2, NBLK, C)
        in_maps.append({
            "xcp": np.ascontiguousarray(xb),
            "bnd": np.ascontiguousarray(
                bands[r0 // 2:(r0 + ROWS) // 2].reshape(
                    ROWS // 2, 128, BFREE)),
        })
    return in_maps


def _unblock(arr):
    """[ROWS, BLK, NBLK, C] blocked -> [ROWS, W, C] f32."""
    return arr.astype(np.float32).transpose(0, 2, 1, 3).reshape(
        ROWS, NBLK * BLK, C)[:, :W]


def kernel(x0, weights, cnts):
    x0 = np.asarray(x0, np.float32)
    weights = np.asarray(weights, np.float32)
    cnts = np.asarray(cnts, np.float32)
    nc = _get_nc()
    in_maps = _host_prep(x0, weights, cnts)
    res = run_bass_kernel_spmd(nc, in_maps, core_ids=list(range(NCORES)))
    return np.ascontiguousarray(np.concatenate(
        [_unblock(res.results[k]["out"]) for k in range(NCORES)], axis=0))


# revision 10
# speedup vs baseline: 1.4220x; 1.0304x over previous
"""Trainium2 Bass kernel for nn_CellAnnotator (per-pixel 8x8 locally-connected
weighted pooling with normalization), SPMD across 8 NeuronCores.

Contract: kernel(**inputs) takes FULL inputs (x0 [512,512,128] f32,
weights [512,512,64] f32, cnts [512,512,1] f32) and returns the FULL
output [512,512,128] f32.

Sharding: rows (H) split across 8 cores, 64 output rows each; each core's
input shard carries a 3+4-row halo (built host-side, zero-padded at the
image borders), so no device-to-device communication is needed.

v2 design:
- The normalization denominator att(cnts, weights) depends only on the
  inputs, so it is folded into the weights host-side:
  w' = w / (att(cnts, w) + 1e-6); then out = att(x0, w') exactly.
  The device runs pure pooling: no cnts channel, no reciprocal/multiply.
- Banded matmul on the TensorEngine: for output row i and a 57-pixel
  column block, the 64-tap sum is PSUM-accumulated over input-row pairs:
  K = 128 = (2 rows) x (64 input cols); lhsT is a banded [128, 57] w'
  tile; rhs is the bf16 input-pair tile [128, 128].
- Even-pair x sharing: x pair tiles exist only for even ri (rows ri,ri+1),
  so each input row is loaded once (plus the 64/57 column overlap). Even
  output rows accumulate 4 pairs; odd rows 5 pairs (the first/last pair
  contributes one row; its other half of the band is zero).
- DMA batching: x tiles hold 2 even pairs (18 DMAs on the sync queue);
  band tiles hold one even+odd row pair of bands, 4+5 pair-slots
  (32 DMAs on the gpsimd queue); outputs are staged 4 rows per tile
  (16 DMAs on the scalar queue).
"""

import numpy as np
import ml_dtypes
from contextlib import ExitStack

import concourse.bass as bass
import concourse.bacc as bacc
import concourse.mybir as mybir
import concourse.tile as tile
from concourse.ap import AP
from concourse.bass_utils import run_bass_kernel_spmd

BF16 = np.dtype(ml_dtypes.bfloat16)
BF8E3 = np.dtype(ml_dtypes.float8_e3m4)   # E3M4: 4 mantissa bits, max 15.5

# Problem constants (hardcoded per contract)
H, W, C = 512, 512, 128
ROI = 8
PAD_LO, PAD_HI = 3, 4          # XLA SAME padding for even kernel
NCORES = 8
ROWS = H // NCORES             # 64 output rows per core
IN_ROWS = ROWS + ROI           # 72 input rows (halo + even-pair pad)
WPAD = W + ROI                 # padded width: cols -3 .. 516 (520)

BLK = 57                       # output pixels per column block
NBLK = 9                       # 8*57 + 56 = 512
NXT = 18                       # x tiles: 2 even pairs each (ri = 4t, 4t+2)
SLOTS = 9                      # band pair-slots per (even,odd) row pair: 4+5
BFREE = SLOTS * NBLK * BLK     # band free elems per row-pair line (4617)

_CACHE = {}


def _build_nc(rep=1):
    f32 = mybir.dt.float32
    bf = mybir.dt.bfloat16
    f8 = mybir.dt.float8e3
    nc = bacc.Bacc("TRN2", target_bir_lowering=False, debug=False,
                   num_devices=NCORES)
    # x pair tiles (even ri only), 2 pairs per line:
    # xcp[t, 64g+u, e, b, c] = xpad[4t+2e+g, 57b+u, c]
    xcp = nc.dram_tensor("xcp", [NXT, 128, 2, NBLK, C], bf,
                         kind="ExternalInput")
    # bands for row pair (2r, 2r+1): 4 even slots + 5 odd slots
    bnd = nc.dram_tensor("bnd", [ROWS // 2, 128, BFREE], f8,
                         kind="ExternalInput")
    # blocked output layout: [row, jj, b, c]; de-blocked + upcast host-side
    out = nc.dram_tensor("out", [ROWS, BLK, NBLK, C], bf,
                         kind="ExternalOutput")

    with tile.TileContext(nc) as tc:
        with ExitStack() as ctx:
            if rep > 1:
                ctx.enter_context(tc.For_i(0, rep, 1))
            xpool = ctx.enter_context(tc.tile_pool(name="xp", bufs=8))
            bpool = ctx.enter_context(tc.tile_pool(name="bp", bufs=8))
            ppool = ctx.enter_context(
                tc.tile_pool(name="pp", bufs=8, space="PSUM"))
            opool = ctx.enter_context(tc.tile_pool(name="op", bufs=3))

            xcache = {}

            def get_x(t):
                """Two even input pairs (ri = 4t, 4t+2), all column blocks:
                [128 = 2x64 positions, 2 pairs, 9 blocks, 128 channels]."""
                if t not in xcache:
                    xt = xpool.tile([128, 2, NBLK, C], bf, tag="xt")
                    fr = 2 * NBLK * C
                    src = AP(xcp, t * 128 * fr, [[fr, 128], [1, fr]])
                    nc.sync.dma_start(xt[:], src)
                    xcache[t] = xt
                return xcache[t]

            OBATCH = 4
            fo = NBLK * C
            otile = None
            btile = None
            for il in range(ROWS):
                if il % 2 == 0:
                    btile = bpool.tile([128, SLOTS, NBLK, BLK], f8, tag="bt")
                    nc.gpsimd.dma_start(
                        btile[:],
                        AP(bnd, (il // 2) * 128 * BFREE,
                           [[BFREE, 128], [1, BFREE]]))
                if il % OBATCH == 0:
                    otile = opool.tile([BLK, OBATCH, fo], bf, tag="ot")
                if il % 2 == 0:
                    pairs = [il, il + 2, il + 4, il + 6]
                    slot0 = 0
                else:
                    pairs = [il - 1, il + 1, il + 3, il + 5, il + 7]
                    slot0 = 4
                for b in range(NBLK):
                    m = W - BLK * b if b == NBLK - 1 else BLK
                    psum = ppool.tile([BLK, C], f32, tag="ps")
                    for s, ri in enumerate(pairs):
                        xt = get_x(ri // 4)
                        nc.tensor.matmul(
                            psum[:m, :], btile[:, slot0 + s, b, :m],
                            xt[:, (ri % 4) // 2, b, :],
                            start=(s == 0), stop=(s == len(pairs) - 1))
                    odst = otile[:m, il % OBATCH, b * C:(b + 1) * C]
                    if b % 3 == 2:
                        nc.scalar.activation(
                            odst, psum[:m, :],
                            mybir.ActivationFunctionType.Identity)
                    else:
                        nc.vector.tensor_copy(odst, psum[:m, :])
                if il % OBATCH == OBATCH - 1:
                    dst = AP(out, (il - OBATCH + 1) * BLK * fo,
                             [[fo, BLK], [BLK * fo, OBATCH], [1, fo]])
                    nc.scalar.dma_start(dst, otile[:])
    nc.compile()
    return nc


def _get_nc(rep=1, variant="full"):
    key = ("nc", rep, variant)
    if key not in _CACHE:
        _CACHE[key] = _build_nc(rep)
    return _CACHE[key]


def _total_cnts(weights, cnts):
    """att(cnts, weights): [H, W] pooled counts (reference padding)."""
    cp = np.zeros((H + ROI - 1, W + ROI - 1), np.float32)
    cp[PAD_LO:PAD_LO + H, PAD_LO:PAD_LO + W] = cnts[:, :, 0]
    wq = weights.reshape(H, W, ROI, ROI)
    tot = np.zeros((H, W), np.float32)
    for p in range(ROI):
        for q in range(ROI):
            tot += cp[p:p + H, q:q + W] * wq[:, :, p, q]
    return tot


def _build_bands(wn):
    """Banded layout of prenormalized weights, 2 output rows per line.

    bands[r, kpos, slot, b, jj] = wn[i, 57b+jj, p, d] at kpos = 64g+jj+d,
    where for even row i=2r: slot in [0,4), p = 2*slot + g;
    for odd row i=2r+1: slot in [4,9), p = 2*(slot-4) - 1 + g, skipping
    p outside [0,8). Zero elsewhere.
    """
    wq = wn.reshape(H, W, ROI, ROI)
    bands = np.zeros((H // 2, 128, SLOTS, NBLK, BLK), BF8E3)
    for b in range(NBLK):
        m = W - BLK * b if b == NBLK - 1 else BLK
        jv = np.arange(m)
        for parity in (0, 1):
            rows = np.arange(parity, H, 2)
            wb = wq[rows, BLK * b:BLK * b + m]     # [H/2, m, 8, 8]
            nslot = 4 if parity == 0 else 5
            for s in range(nslot):
                for g in range(2):
                    p = 2 * s + g if parity == 0 else 2 * s - 1 + g
                    if p < 0 or p >= ROI:
                        continue
                    slot = s if parity == 0 else 4 + s
                    for d in range(ROI):
                        bands[:, 64 * g + jv + d, slot, b, jv] = \
                            wb[:, jv, p, d].astype(BF8E3)
    return bands


def _host_prep(x0, weights, cnts):
    tot = _total_cnts(weights, cnts) + 1e-6
    wn = weights / tot[:, :, None]
    # scale bands into E3M4's sweet range [~2^-6, 15.5]; fold the inverse
    # scale into x via an exact bf16 exponent shift
    k = int(np.floor(np.log2(15.5 / wn.max())))
    wn = wn * np.float32(2.0 ** k)
    x0 = x0 * np.float32(2.0 ** -k)

    # padded image: row pr holds x0[pr-3]; rows 0-2 and 515-519 zero.
    # Output global row gi reads padded rows gi..gi+7; core k's local input
    # row lr is padded row r0+lr, lr in [0, 72).
    xpad = np.zeros((H + ROI, WPAD, C), BF16)
    xpad[PAD_LO:PAD_LO + H, PAD_LO:PAD_LO + W, :] = x0.astype(BF16)
    bands = _build_bands(wn)
    in_maps = []
    for k in range(NCORES):
        r0 = k * ROWS
        # xcp[t, 64g+u, e, b, c] = xloc[4t+2e+g, 57b+u, c]
        xloc = xpad[r0:r0 + IN_ROWS]
        s0, s1, s2 = xloc.strides
        xb = np.lib.stride_tricks.as_strided(
            xloc, shape=(NXT, 2, 64, 2, NBLK, C),
            strides=(4 * s0, s0, s1, 2 * s0, BLK * s1, s2))
        # xb[t, g, u, e, b, c] -> want [t, 64g+u, e, b, c]
        xb = xb.reshape(NXT, 128, 2, NBLK, C)
        in_maps.append({
            "xcp": np.ascontiguousarray(xb),
            "bnd": np.ascontiguousarray(
                bands[r0 // 2:(r0 + ROWS) // 2].reshape(
                    ROWS // 2, 128, BFREE)),
        })
    return in_maps


def _unblock(arr):
    """[ROWS, BLK, NBLK, C] blocked -> [ROWS, W, C] f32."""
    return arr.astype(np.float32).transpose(0, 2, 1, 3).reshape(
        ROWS, NBLK * BLK, C)[:, :W]


def kernel(x0, weights, cnts):
    x0 = np.asarray(x0, np.float32)
    weights = np.asarray(weights, np.float32)
    cnts = np.asarray(cnts, np.float32)
    nc = _get_nc()
    in_maps = _host_prep(x0, weights, cnts)
    res = run_bass_kernel_spmd(nc, in_maps, core_ids=list(range(NCORES)))
    return np.ascontiguousarray(np.concatenate(
        [_unblock(res.results[k]["out"]) for k in range(NCORES)], axis=0))


# revision 15
# speedup vs baseline: 1.4253x; 1.0023x over previous
"""Trainium2 Bass kernel for nn_CellAnnotator (per-pixel 8x8 locally-connected
weighted pooling with normalization), SPMD across 8 NeuronCores.

Contract: kernel(**inputs) takes FULL inputs (x0 [512,512,128] f32,
weights [512,512,64] f32, cnts [512,512,1] f32) and returns the FULL
output [512,512,128] f32.

Sharding: rows (H) split across 8 cores, 64 output rows each; each core's
input shard carries a 3+4-row halo (built host-side, zero-padded at the
image borders), so no device-to-device communication is needed.

v2 design:
- The normalization denominator att(cnts, weights) depends only on the
  inputs, so it is folded into the weights host-side:
  w' = w / (att(cnts, w) + 1e-6); then out = att(x0, w') exactly.
  The device runs pure pooling: no cnts channel, no reciprocal/multiply.
- Banded matmul on the TensorEngine: for output row i and a 57-pixel
  column block, the 64-tap sum is PSUM-accumulated over input-row pairs:
  K = 128 = (2 rows) x (64 input cols); lhsT is a banded [128, 57] w'
  tile; rhs is the bf16 input-pair tile [128, 128].
- Even-pair x sharing: x pair tiles exist only for even ri (rows ri,ri+1),
  so each input row is loaded once (plus the 64/57 column overlap). Even
  output rows accumulate 4 pairs; odd rows 5 pairs (the first/last pair
  contributes one row; its other half of the band is zero).
- DMA batching: x tiles hold 2 even pairs (18 DMAs on the sync queue);
  band tiles hold one even+odd row pair of bands, 4+5 pair-slots
  (32 DMAs on the gpsimd queue); outputs are staged 4 rows per tile
  (16 DMAs on the scalar queue).
"""

import numpy as np
import ml_dtypes
from contextlib import ExitStack

import concourse.bass as bass
import concourse.bacc as bacc
import concourse.mybir as mybir
import concourse.tile as tile
from concourse.ap import AP
from concourse.bass_utils import run_bass_kernel_spmd

BF16 = np.dtype(ml_dtypes.bfloat16)
BF8E3 = np.dtype(ml_dtypes.float8_e3m4)   # E3M4: 4 mantissa bits, max 15.5

# Problem constants (hardcoded per contract)
H, W, C = 512, 512, 128
ROI = 8
PAD_LO, PAD_HI = 3, 4          # XLA SAME padding for even kernel
NCORES = 8
ROWS = H // NCORES             # 64 output rows per core
IN_ROWS = ROWS + ROI           # 72 input rows (halo + even-pair pad)
WPAD = W + ROI                 # padded width: cols -3 .. 516 (520)

BLK = 57                       # output pixels per column block
NBLK = 9                       # 8*57 + 56 = 512
NXT = 18                       # x tiles: 2 even pairs each (ri = 4t, 4t+2)
SLOTS = 9                      # band pair-slots per (even,odd) row pair: 4+5
BFREE = SLOTS * NBLK * BLK     # band free elems per row-pair line (4617)

_CACHE = {}


def _build_nc(rep=1):
    f32 = mybir.dt.float32
    bf = mybir.dt.bfloat16
    f8 = mybir.dt.float8e3
    nc = bacc.Bacc("TRN2", target_bir_lowering=False, debug=False,
                   num_devices=NCORES)
    # x pair tiles (even ri only), 2 pairs per line:
    # xcp[t, 64g+u, e, b, c] = xpad[4t+2e+g, 57b+u, c]
    xcp = nc.dram_tensor("xcp", [NXT, 128, 2, NBLK, C], bf,
                         kind="ExternalInput")
    # bands for row pair (2r, 2r+1): 4 even slots + 5 odd slots
    bnd = nc.dram_tensor("bnd", [ROWS // 2, 128, BFREE], f8,
                         kind="ExternalInput")
    # blocked output layout: [row, jj, b, c]; de-blocked + upcast host-side
    out = nc.dram_tensor("out", [ROWS, BLK, NBLK, C], bf,
                         kind="ExternalOutput")

    with tile.TileContext(nc) as tc:
        with ExitStack() as ctx:
            if rep > 1:
                ctx.enter_context(tc.For_i(0, rep, 1))
            xpool = ctx.enter_context(tc.tile_pool(name="xp", bufs=8))
            bpool = ctx.enter_context(tc.tile_pool(name="bp", bufs=8))
            ppool = ctx.enter_context(
                tc.tile_pool(name="pp", bufs=8, space="PSUM"))
            opool = ctx.enter_context(tc.tile_pool(name="op", bufs=3))

            xcache = {}

            def get_x(t):
                """Two even input pairs (ri = 4t, 4t+2), all column blocks:
                [128 = 2x64 positions, 2 pairs, 9 blocks, 128 channels]."""
                if t not in xcache:
                    xt = xpool.tile([128, 2, NBLK, C], bf, tag="xt")
                    fr = 2 * NBLK * C
                    src = AP(xcp, t * 128 * fr, [[fr, 128], [1, fr]])
                    nc.sync.dma_start(xt[:], src)
                    xcache[t] = xt
                return xcache[t]

            OBATCH = 4
            fo = NBLK * C
            otile = None
            btile = None
            for il in range(ROWS):
                if il % 2 == 0:
                    btile = bpool.tile([128, SLOTS, NBLK, BLK], f8, tag="bt")
                    nc.gpsimd.dma_start(
                        btile[:],
                        AP(bnd, (il // 2) * 128 * BFREE,
                           [[BFREE, 128], [1, BFREE]]))
                if il % OBATCH == 0:
                    otile = opool.tile([BLK, OBATCH, fo], bf, tag="ot")
                if il % 2 == 0:
                    pairs = [il, il + 2, il + 4, il + 6]
                    slot0 = 0
                else:
                    pairs = [il - 1, il + 1, il + 3, il + 5, il + 7]
                    slot0 = 4
                for b in range(NBLK):
                    m = W - BLK * b if b == NBLK - 1 else BLK
                    psum = ppool.tile([BLK, C], f32, tag="ps")
                    for s, ri in enumerate(pairs):
                        xt = get_x(ri // 4)
                        nc.tensor.matmul(
                            psum[:m, :], btile[:, slot0 + s, b, :m],
                            xt[:, (ri % 4) // 2, b, :],
                            start=(s == 0), stop=(s == len(pairs) - 1))
                    odst = otile[:m, il % OBATCH, b * C:(b + 1) * C]
                    if b % 3 == 2:
                        nc.scalar.activation(
                            odst, psum[:m, :],
                            mybir.ActivationFunctionType.Identity)
                    else:
                        nc.vector.tensor_copy(odst, psum[:m, :])
                if il % OBATCH == OBATCH - 1:
                    dst = AP(out, (il - OBATCH + 1) * BLK * fo,
                             [[fo, BLK], [BLK * fo, OBATCH], [1, fo]])
                    nc.scalar.dma_start(dst, otile[:])
    nc.compile()
    return nc


def _get_nc(rep=1, variant="full"):
    key = ("nc", rep, variant)
    if key not in _CACHE:
        _CACHE[key] = _build_nc(rep)
    return _CACHE[key]


def _total_cnts(weights, cnts):
    """att(cnts, weights): [H, W] pooled counts (reference padding)."""
    cp = np.zeros((H + ROI - 1, W + ROI - 1), np.float32)
    cp[PAD_LO:PAD_LO + H, PAD_LO:PAD_LO + W] = cnts[:, :, 0]
    wq = weights.reshape(H, W, ROI, ROI)
    tot = np.zeros((H, W), np.float32)
    for p in range(ROI):
        for q in range(ROI):
            tot += cp[p:p + H, q:q + W] * wq[:, :, p, q]
    return tot


def _build_bands(wn):
    """Banded layout of prenormalized weights, 2 output rows per line.

    bands[r, kpos, slot, b, jj] = wn[i, 57b+jj, p, d] at kpos = 64g+jj+d,
    where for even row i=2r: slot in [0,4), p = 2*slot + g;
    for odd row i=2r+1: slot in [4,9), p = 2*(slot-4) - 1 + g, skipping
    p outside [0,8). Zero elsewhere.
    """
    wq = wn.reshape(H, W, ROI, ROI)
    bands = np.zeros((H // 2, 128, SLOTS, NBLK, BLK), BF8E3)
    for b in range(NBLK):
        m = W - BLK * b if b == NBLK - 1 else BLK
        jv = np.arange(m)
        for parity in (0, 1):
            rows = np.arange(parity, H, 2)
            wb = wq[rows, BLK * b:BLK * b + m]     # [H/2, m, 8, 8]
            nslot = 4 if parity == 0 else 5
            for s in range(nslot):
                for g in range(2):
                    p = 2 * s + g if parity == 0 else 2 * s - 1 + g
                    if p < 0 or p >= ROI:
                        continue
                    slot = s if parity == 0 else 4 + s
                    for d in range(ROI):
                        bands[:, 64 * g + jv + d, slot, b, jv] = \
                            wb[:, jv, p, d].astype(BF8E3)
    return bands


def _host_prep(x0, weights, cnts):
    tot = _total_cnts(weights, cnts) + 1e-6
    wn = weights / tot[:, :, None]
    # scale bands into E3M4's sweet range [~2^-6, 15.5]; fold the inverse
    # scale into x via an exact bf16 exponent shift
    k = int(np.floor(np.log2(15.5 / wn.max())))
    wn = wn * np.float32(2.0 ** k)
    x0 = x0 * np.float32(2.0 ** -k)

    # padded image: row pr holds x0[pr-3]; rows 0-2 and 515-519 zero.
    # Output global row gi reads padded rows gi..gi+7; core k's local input
    # row lr is padded row r0+lr, lr in [0, 72).
    xpad = np.zeros((H + ROI, WPAD, C), BF16)
    xpad[PAD_LO:PAD_LO + H, PAD_LO:PAD_LO + W, :] = x0.astype(BF16)
    bands = _build_bands(wn)
    in_maps = []
    for k in range(NCORES):
        r0 = k * ROWS
        # xcp[t, 64g+u, e, b, c] = xloc[4t+2e+g, 57b+u, c]
        xloc = xpad[r0:r0 + IN_ROWS]
        s0, s1, s2 = xloc.strides
        xb = np.lib.stride_tricks.as_strided(
            xloc, shape=(NXT, 2, 64, 2, NBLK, C),
            strides=(4 * s0, s0, s1, 2 * s0, BLK * s1, s2))
        # xb[t, g, u, e, b, c] -> want [t, 64g+u, e, b, c]
        xb = xb.reshape(NXT, 128, 2, NBLK, C)
        in_maps.append({
            "xcp": np.ascontiguousarray(xb),
            "bnd": np.ascontiguousarray(
                bands[r0 // 2:(r0 + ROWS) // 2].reshape(
                    ROWS // 2, 128, BFREE)),
        })
    return in_maps


def _unblock(arr):
    """[ROWS, BLK, NBLK, C] blocked -> [ROWS, W, C] f32."""
    return arr.astype(np.float32).transpose(0, 2, 1, 3).reshape(
        ROWS, NBLK * BLK, C)[:, :W]


def kernel(x0, weights, cnts):
    x0 = np.asarray(x0, np.float32)
    weights = np.asarray(weights, np.float32)
    cnts = np.asarray(cnts, np.float32)
    nc = _get_nc()
    in_maps = _host_prep(x0, weights, cnts)
    res = run_bass_kernel_spmd(nc, in_maps, core_ids=list(range(NCORES)))
    return np.ascontiguousarray(np.concatenate(
        [_unblock(res.results[k]["out"]) for k in range(NCORES)], axis=0))
